# revision 17
# baseline (speedup 1.0000x reference)
"""NNUE (HalfKP embedding + tiny MLP) Trainium2 kernel.

Strategy (hardcoded for B=4096, H=20480, D=40960, 8 cores):
  - Pure batch data-parallel: each core handles 512 samples. No collectives.
  - Host prep: cast 0/1 activations to fp8-e4m3 (exact) and transpose to
    feature-major; build a combined feature-transform weight matrix
    Wt [DP, 512] where Wt[d, 0:256] / Wt[d, 256:512] are the Ww / Wb columns
    that multiply concat([white, black])[d] for the w256 / b256 accumulators.
    Weights are quantized to fp8-e4m3 with a per-output-column scale; the
    biases ride along as one extra all-ones act row. Both streams are stored
    partition-major ([128, NKT, cols]) so each DMA descriptor run is
    multi-KB contiguous.
  - Device: stream Wt and actT through SBUF; DoubleRow fp8 matmuls (2
    k-planes/cycle) accumulate x^T = [w256; b256]^T into 4 psum tiles.
    Dequant + pov-select + relu (DVE/ACT split), then the 512->32->32->1
    MLP in fp32. Output [1, 512] per core.
"""

import numpy as np
import ml_dtypes

B = 4096
H = 20480
D = 2 * H
NCORES = 8
BC = B // NCORES  # 512 samples per core
KT = 128          # contraction tile (partition dim)
NKT = D // KT + 1  # 320 k-tiles + 1 bias tile (act row of ones)
DP = NKT * KT      # padded contraction dim (41088)
G = 16             # k-tiles per DMA chunk
NG = (NKT + G - 1) // G  # 21 chunks (last one partial: 1 k-tile)

bf16 = ml_dtypes.bfloat16
f8 = ml_dtypes.float8_e4m3fn
F8MAX = 240.0  # TRN FP8_EXP4 max normal is +-240 (not OCP's 448)

TRACE = False
LAST_EXEC_NS = None
LAST_RESULTS = None

_COMPILED = None


def _prune_redundant_dma_waits(nc, mybir):
    """Drop transitively-implied waits from DMA instructions.

    The DMA DIRECT2D descriptor has a single sync-wait slot, but Tile's sem
    assignment is not transitively minimal: a streaming-load DMA that recycles
    a buffer slot carries both a WAR wait on the consumer engine (e.g. PE) and
    a WAW wait on its DMA-lane sem, even though the consumers themselves
    waited on that lane sem (so consumer-done implies lane-value reached).

    We compute a transitive vector clock per instruction: waiting (S >= v)
    implies everything the updater that brings S to v happened-after (same
    in-order assumption per sem lane that Tile's own WAW logic relies on).
    A wait on a DMA is dropped when the join of its remaining waits already
    guarantees it.
    """
    from collections import defaultdict

    f = nc.m.functions[0]
    insts = [i for b in f.blocks for i in b.instructions]

    def is_dma(i):
        return "dma" in type(i).__name__.lower()

    def wait_list(i):
        si = getattr(i, "sync_info", None)
        if si is None:
            return []
        return [
            (w.ant_name, w.wait_value)
            for w in si.on_wait
            if w.wait_mode == "sem-ge-imm" and w.wait_value is not None
        ]

    def update_list(i):
        si = getattr(i, "sync_info", None)
        if si is None:
            return []
        out = []
        for u in si.on_update:
            if u.update_mode == "sem-add-imm" and u.update_value is not None:
                out.append((u.ant_name, u.update_value))
            elif u.update_mode == "sem-inc":
                out.append((u.ant_name, 1))
            else:
                out.append((u.ant_name, None))  # non-monotonic: poisons sem
        return out

    sem_hist = defaultdict(list)  # sem -> [(cum_after, clock)] in order
    poisoned = set()
    cum = defaultdict(int)
    eng_clock = {}  # per-engine program-order running clock

    def join(a, b):
        if not b:
            return a
        out = dict(a)
        for k, v in b.items():
            if out.get(k, -1) < v:
                out[k] = v
        return out

    def clock_at(sem, val):
        if sem in poisoned:
            return None
        hist = sem_hist.get(sem)
        if not hist:
            return None
        lo, hi = 0, len(hist)
        while lo < hi:
            mid = (lo + hi) // 2
            if hist[mid][0] < val:
                lo = mid + 1
            else:
                hi = mid
        if lo == len(hist):
            return None
        return hist[lo][1]

    for i in insts:
        c = {}
        eng = getattr(i, "engine", None)
        if not is_dma(i) and eng is not None and eng in eng_clock:
            c = dict(eng_clock[eng])
        for sem, val in wait_list(i):
            wc = clock_at(sem, val)
            if wc is not None:
                c = join(c, wc)
            if c.get(sem, -1) < val:
                c[sem] = val
        for sem, inc in update_list(i):
            if inc is None:
                poisoned.add(sem)
                continue
            cum[sem] += inc
            c = join(c, {sem: cum[sem]})
            sem_hist[sem].append((cum[sem], c))
        if not is_dma(i) and eng is not None:
            eng_clock[eng] = c

    n_dropped = 0
    for i in insts:
        if not is_dma(i):
            continue
        si = getattr(i, "sync_info", None)
        if si is None or len(si.on_wait) <= 1:
            continue
        kept = list(si.on_wait)
        for w in list(kept):
            if len(kept) <= 1:
                break
            if w.wait_mode != "sem-ge-imm" or w.wait_value is None:
                continue
            others = {}
            ok = True
            for o in kept:
                if o is w:
                    continue
                if o.wait_mode != "sem-ge-imm" or o.wait_value is None:
                    ok = False
                    break
                oc = clock_at(o.ant_name, o.wait_value)
                if oc is None:
                    ok = False
                    break
                others = join(others, oc)
            if ok and others.get(w.ant_name, -1) >= w.wait_value:
                kept.remove(w)
                n_dropped += 1
        if len(kept) != len(si.on_wait):
            i.sync_info = mybir.SyncInfo(on_wait=kept, on_update=list(si.on_update))
    return n_dropped


def _build():
    import concourse.bacc as bacc
    import concourse.mybir as mybir
    import concourse.tile as tile
    from concourse.bass import ts

    fp32 = mybir.dt.float32
    f8t = mybir.dt.float8e4

    nc = bacc.Bacc("TRN2", target_bir_lowering=False, debug=False)

    actT = nc.dram_tensor("actT", (128, NKT, BC), f8t, kind="ExternalInput").ap()
    wt = nc.dram_tensor("wt", (128, NKT, 512), f8t, kind="ExternalInput").ap()
    povT = nc.dram_tensor("povT", (128, BC), fp32, kind="ExternalInput").ap()
    scales = nc.dram_tensor("scales", (128, 4), fp32, kind="ExternalInput").ap()
    w0t = nc.dram_tensor("w0t", (512, 32), fp32, kind="ExternalInput").ap()
    w1t = nc.dram_tensor("w1t", (32, 32), fp32, kind="ExternalInput").ap()
    w2t = nc.dram_tensor("w2t", (32, 1), fp32, kind="ExternalInput").ap()
    b0 = nc.dram_tensor("b0", (32, 1), fp32, kind="ExternalInput").ap()
    b1 = nc.dram_tensor("b1", (32, 1), fp32, kind="ExternalInput").ap()
    b2 = nc.dram_tensor("b2", (1, 1), fp32, kind="ExternalInput").ap()
    out = nc.dram_tensor("out", (1, BC), fp32, kind="ExternalOutput").ap()

    relu = mybir.ActivationFunctionType.Relu
    ident = mybir.ActivationFunctionType.Identity
    copyf = mybir.ActivationFunctionType.Copy
    dr = mybir.MatmulPerfMode.DoubleRow

    with tile.TileContext(nc) as tc:
        with (
            tc.tile_pool(name="consts", bufs=1) as cp,
            tc.tile_pool(name="acts", bufs=8) as ap_,
            tc.tile_pool(name="wts", bufs=8) as wp,
            tc.tile_pool(name="xs", bufs=1) as xp,
            tc.tile_pool(name="tmps", bufs=2) as tp,
            tc.tile_pool(name="psum", bufs=1, space="PSUM") as pp,
        ):
            # constants
            povT_s = cp.tile([128, BC], fp32, tag="povT")
            nc.sync.dma_start(povT_s[:], povT)
            scales_s = cp.tile([128, 4], fp32, tag="scales")
            nc.sync.dma_start(scales_s[:], scales)
            w0t_s = cp.tile([128, 4, 32], fp32, tag="w0t")
            nc.sync.dma_start(w0t_s[:], w0t.rearrange("(a p) m -> p a m", p=128))
            w1t_s = cp.tile([32, 32], fp32, tag="w1t")
            nc.sync.dma_start(w1t_s[:], w1t)
            w2t_s = cp.tile([32, 1], fp32, tag="w2t")
            nc.sync.dma_start(w2t_s[:], w2t)
            b0_s = cp.tile([32, 1], fp32, tag="b0")
            nc.sync.dma_start(b0_s[:], b0)
            b1_s = cp.tile([32, 1], fp32, tag="b1")
            nc.sync.dma_start(b1_s[:], b1)
            b2_s = cp.tile([1, 1], fp32, tag="b2")
            nc.sync.dma_start(b2_s[:], b2)

            # PE warm-up during the first stream-DMA window: junk fp32
            # matmuls trip the HAM clock gate to 2.4GHz before the real
            # accumulation starts (~3.4us of sustained work required).
            warm = pp.tile([128, BC], fp32, tag="warm")
            for _ in range(4):
                nc.tensor.matmul(
                    warm[:], povT_s[:, 0:128], povT_s[:], start=True, stop=True
                )

            # psum accumulators: x^T halves [features 128, batch 512]
            # 0: w256[0:128], 1: w256[128:256], 2: b256[0:128], 3: b256[128:256]
            # (biases are folded in via the final all-ones act k-tile)
            acc = [
                pp.tile([128, BC], fp32, tag=f"acc{j}", name=f"acc{j}")
                for j in range(4)
            ]

            # main streaming loop over contraction dim; fp8 DoubleRow
            # consumes k-tile pairs (2 k-planes per cycle).
            kt_done = 0
            for g in range(NG):
                g0 = g * G
                gsz = min(G, NKT - g0)
                at = ap_.tile([128, G, BC], f8t, tag="at")
                nc.sync.dma_start(at[:, :gsz, :], actT[:, g0 : g0 + gsz, :])
                wtt = wp.tile([128, G, 512], f8t, tag="wtt")
                nc.sync.dma_start(wtt[:, :gsz, :], wt[:, g0 : g0 + gsz, :])
                i = 0
                while i < gsz:
                    first = kt_done == 0
                    if i + 2 <= gsz:
                        last = kt_done + 2 == NKT
                        for j in range(4):
                            nc.tensor.matmul(
                                acc[j][:],
                                wtt[:, i : i + 2, ts(j, 128)],
                                at[:, i : i + 2, :],
                                start=first,
                                stop=last,
                                perf_mode=dr,
                            )
                        kt_done += 2
                        i += 2
                    else:
                        last = kt_done + 1 == NKT
                        for j in range(4):
                            nc.tensor.matmul(
                                acc[j][:],
                                wtt[:, i, ts(j, 128)],
                                at[:, i, :],
                                start=first,
                                stop=last,
                            )
                        kt_done += 1
                        i += 1

            # dequant + pov select + relu, feature-major.
            # x_top = pov ? w256 : b256 ; x_bot = pov ? b256 : w256
            xs = [
                xp.tile([128, BC], fp32, tag=f"x{a}", name=f"x{a}")
                for a in range(4)
            ]
            for i in range(2):
                aw = tp.tile([128, BC], fp32, tag="aw")
                nc.scalar.activation(
                    aw[:], acc[i][:], copyf, scale=scales_s[:, i : i + 1]
                )
                ab = tp.tile([128, BC], fp32, tag="ab")
                nc.scalar.activation(
                    ab[:], acc[2 + i][:], copyf, scale=scales_s[:, 2 + i : 3 + i]
                )
                dd = tp.tile([128, BC], fp32, tag="dd")
                nc.vector.tensor_sub(dd[:], aw[:], ab[:])
                pd = tp.tile([128, BC], fp32, tag="pd")
                nc.vector.tensor_mul(pd[:], dd[:], povT_s[:])
                xt = tp.tile([128, BC], fp32, tag="xt")
                nc.vector.tensor_add(xt[:], ab[:], pd[:])
                nc.scalar.activation(xs[i][:], xt[:], relu)
                xb = tp.tile([128, BC], fp32, tag="xb")
                nc.vector.tensor_sub(xb[:], aw[:], pd[:])
                nc.vector.tensor_relu(xs[2 + i][:], xb[:])

            # MLP: 512 -> 32 -> 32 -> 1, fp32
            h0 = pp.tile([32, BC], fp32, tag="h0")
            for a in range(4):
                nc.tensor.matmul(
                    h0[:], w0t_s[:, a, :], xs[a][:], start=(a == 0), stop=(a == 3)
                )
            h0s = tp.tile([32, BC], fp32, tag="h0s")
            nc.scalar.activation(h0s[:], h0[:], relu, bias=b0_s[:])

            h1 = pp.tile([32, BC], fp32, tag="h1")
            nc.tensor.matmul(h1[:], w1t_s[:], h0s[:], start=True, stop=True)
            h1s = tp.tile([32, BC], fp32, tag="h1s")
            nc.scalar.activation(h1s[:], h1[:], relu, bias=b1_s[:])

            y = pp.tile([1, BC], fp32, tag="y")
            nc.tensor.matmul(y[:], w2t_s[:], h1s[:], start=True, stop=True)
            ys = tp.tile([1, BC], fp32, tag="ys")
            nc.scalar.activation(ys[:], y[:], ident, bias=b2_s[:])

            nc.sync.dma_start(out, ys[:])

    _prune_redundant_dma_waits(nc, mybir)
    nc.compile()
    return nc


def _get_compiled():
    global _COMPILED
    if _COMPILED is None:
        _COMPILED = _build()
    return _COMPILED


def kernel(pov, white, black, Ww, bw, Wb, bb, W0, b0, W1, b1, W2, b2):
    global LAST_EXEC_NS, LAST_RESULTS
    from concourse import bass_utils

    pov = np.asarray(pov, np.float32)
    white = np.asarray(white, np.float32)
    black = np.asarray(black, np.float32)
    Ww = np.asarray(Ww, np.float32)
    Wb = np.asarray(Wb, np.float32)

    # Combined feature-transform weights, feature-major [DP, 512].
    # Row D (the all-ones act row) carries the biases.
    Wf = np.zeros((DP, 512), dtype=np.float32)
    Wf[:H, 0:256] = Ww[:, :H].T
    Wf[H:D, 0:256] = Ww[:, H:].T
    Wf[:H, 256:512] = Wb[:, H:].T
    Wf[H:D, 256:512] = Wb[:, :H].T
    Wf[D, 0:256] = np.asarray(bw, np.float32)
    Wf[D, 256:512] = np.asarray(bb, np.float32)

    # fp8 quantization with per-output-column scales
    s = np.abs(Wf).max(axis=0) / F8MAX  # [512]
    s = np.maximum(s, 1e-30)
    Wq = (Wf / s).astype(f8)  # [DP, 512]
    # partition-major layout [128, NKT, 512]
    wt_dev = np.ascontiguousarray(Wq.reshape(NKT, 128, 512).transpose(1, 0, 2))

    scales = np.ascontiguousarray(s.reshape(4, 128).T)  # [128, 4], col j = s[j*128:(j+1)*128]

    whiteb = white.astype(f8)
    blackb = black.astype(f8)

    w0t = np.ascontiguousarray(np.asarray(W0, np.float32).T)  # [512, 32]
    w1t = np.ascontiguousarray(np.asarray(W1, np.float32).T)  # [32, 32]
    w2t = np.ascontiguousarray(np.asarray(W2, np.float32).T)  # [32, 1]
    b0c = np.asarray(b0, np.float32).reshape(32, 1)
    b1c = np.asarray(b1, np.float32).reshape(32, 1)
    b2c = np.asarray(b2, np.float32).reshape(1, 1)

    in_maps = []
    for c in range(NCORES):
        sl = slice(c * BC, (c + 1) * BC)
        actT = np.zeros((DP, BC), dtype=f8)
        actT[:H] = whiteb[sl].T
        actT[H:D] = blackb[sl].T
        actT[D] = 1.0  # bias row
        act_dev = np.ascontiguousarray(
            actT.reshape(NKT, 128, BC).transpose(1, 0, 2)
        )
        povT = np.ascontiguousarray(
            np.broadcast_to(pov[sl].reshape(1, BC), (128, BC))
        )
        in_maps.append(
            {
                "actT": act_dev,
                "wt": wt_dev,
                "povT": povT,
                "scales": scales,
                "w0t": w0t,
                "w1t": w1t,
                "w2t": w2t,
                "b0": b0c,
                "b1": b1c,
                "b2": b2c,
            }
        )

    nc = _get_compiled()
    res = bass_utils.run_bass_kernel_spmd(
        nc, in_maps, core_ids=list(range(NCORES)), trace=TRACE
    )
    LAST_EXEC_NS = res.exec_time_ns
    LAST_RESULTS = res

    y = np.empty((B, 1), np.float32)
    for c in range(NCORES):
        y[c * BC : (c + 1) * BC, 0] = res.results[c]["out"].reshape(BC)
    return y


# revision 26
# speedup vs baseline: 1.0141x; 1.0141x over previous
"""NNUE (HalfKP embedding + tiny MLP) Trainium2 kernel.

Strategy (hardcoded for B=4096, H=20480, D=40960, 8 cores):
  - Pure batch data-parallel: each core handles 512 samples. No collectives.
  - Host prep: cast 0/1 activations to fp8-e4m3 (exact) and transpose to
    feature-major; build a combined feature-transform weight matrix
    Wt [DP, 512] where Wt[d, 0:256] / Wt[d, 256:512] are the Ww / Wb columns
    that multiply concat([white, black])[d] for the w256 / b256 accumulators.
    Weights are quantized to fp8-e4m3 with a per-output-column scale; the
    biases ride along as one extra all-ones act row. Both streams are stored
    partition-major ([128, NKT, cols]) so each DMA descriptor run is
    multi-KB contiguous.
  - Device: stream Wt and actT through SBUF; DoubleRow fp8 matmuls (2
    k-planes/cycle) accumulate x^T = [w256; b256]^T into 4 psum tiles.
    Dequant + pov-select + relu (DVE/ACT split), then the 512->32->32->1
    MLP in fp32. Output [1, 512] per core.
"""

import numpy as np
import ml_dtypes

B = 4096
H = 20480
D = 2 * H
NCORES = 8
BC = B // NCORES  # 512 samples per core
KT = 128          # contraction tile (partition dim)
NKT = D // KT + 1  # 320 k-tiles + 1 bias tile (act row of ones)
DP = NKT * KT      # padded contraction dim (41088)
G = 16             # max k-tiles per DMA chunk
# Chunk size plan: small leading chunks so the PE starts within ~2-3us of
# kernel start (the HAM warm-up window), then full 1MiB chunks.
CHUNKS = [2, 2, 4, 8] + [16] * 19 + [1]
assert sum(CHUNKS) == NKT

bf16 = ml_dtypes.bfloat16
f8 = ml_dtypes.float8_e4m3fn
F8MAX = 240.0  # TRN FP8_EXP4 max normal is +-240 (not OCP's 448)

TRACE = False
LAST_EXEC_NS = None
LAST_RESULTS = None

_COMPILED = None


def _prune_redundant_dma_waits(nc, mybir):
    """Drop transitively-implied waits from DMA instructions.

    The DMA DIRECT2D descriptor has a single sync-wait slot, but Tile's sem
    assignment is not transitively minimal: a streaming-load DMA that recycles
    a buffer slot carries both a WAR wait on the consumer engine (e.g. PE) and
    a WAW wait on its DMA-lane sem, even though the consumers themselves
    waited on that lane sem (so consumer-done implies lane-value reached).

    We compute a transitive vector clock per instruction: waiting (S >= v)
    implies everything the updater that brings S to v happened-after (same
    in-order assumption per sem lane that Tile's own WAW logic relies on).
    A wait on a DMA is dropped when the join of its remaining waits already
    guarantees it.
    """
    from collections import defaultdict

    f = nc.m.functions[0]
    insts = [i for b in f.blocks for i in b.instructions]

    def is_dma(i):
        return "dma" in type(i).__name__.lower()

    def wait_list(i):
        si = getattr(i, "sync_info", None)
        if si is None:
            return []
        return [
            (w.ant_name, w.wait_value)
            for w in si.on_wait
            if w.wait_mode == "sem-ge-imm" and w.wait_value is not None
        ]

    def update_list(i):
        si = getattr(i, "sync_info", None)
        if si is None:
            return []
        out = []
        for u in si.on_update:
            if u.update_mode == "sem-add-imm" and u.update_value is not None:
                out.append((u.ant_name, u.update_value))
            elif u.update_mode == "sem-inc":
                out.append((u.ant_name, 1))
            else:
                out.append((u.ant_name, None))  # non-monotonic: poisons sem
        return out

    sem_hist = defaultdict(list)  # sem -> [(cum_after, clock)] in order
    poisoned = set()
    cum = defaultdict(int)
    eng_clock = {}  # per-engine program-order running clock

    def join(a, b):
        if not b:
            return a
        out = dict(a)
        for k, v in b.items():
            if out.get(k, -1) < v:
                out[k] = v
        return out

    def clock_at(sem, val):
        if sem in poisoned:
            return None
        hist = sem_hist.get(sem)
        if not hist:
            return None
        lo, hi = 0, len(hist)
        while lo < hi:
            mid = (lo + hi) // 2
            if hist[mid][0] < val:
                lo = mid + 1
            else:
                hi = mid
        if lo == len(hist):
            return None
        return hist[lo][1]

    for i in insts:
        c = {}
        eng = getattr(i, "engine", None)
        if not is_dma(i) and eng is not None and eng in eng_clock:
            c = dict(eng_clock[eng])
        for sem, val in wait_list(i):
            wc = clock_at(sem, val)
            if wc is not None:
                c = join(c, wc)
            if c.get(sem, -1) < val:
                c[sem] = val
        for sem, inc in update_list(i):
            if inc is None:
                poisoned.add(sem)
                continue
            cum[sem] += inc
            c = join(c, {sem: cum[sem]})
            sem_hist[sem].append((cum[sem], c))
        if not is_dma(i) and eng is not None:
            eng_clock[eng] = c

    n_dropped = 0
    for i in insts:
        if not is_dma(i):
            continue
        si = getattr(i, "sync_info", None)
        if si is None or len(si.on_wait) <= 1:
            continue
        kept = list(si.on_wait)
        for w in list(kept):
            if len(kept) <= 1:
                break
            if w.wait_mode != "sem-ge-imm" or w.wait_value is None:
                continue
            others = {}
            ok = True
            for o in kept:
                if o is w:
                    continue
                if o.wait_mode != "sem-ge-imm" or o.wait_value is None:
                    ok = False
                    break
                oc = clock_at(o.ant_name, o.wait_value)
                if oc is None:
                    ok = False
                    break
                others = join(others, oc)
            if ok and others.get(w.ant_name, -1) >= w.wait_value:
                kept.remove(w)
                n_dropped += 1
        if len(kept) != len(si.on_wait):
            i.sync_info = mybir.SyncInfo(on_wait=kept, on_update=list(si.on_update))
    return n_dropped


def _build():
    import concourse.bacc as bacc
    import concourse.mybir as mybir
    import concourse.tile as tile
    from concourse.bass import ts

    fp32 = mybir.dt.float32
    f8t = mybir.dt.float8e4
    bft = mybir.dt.bfloat16

    nc = bacc.Bacc("TRN2", target_bir_lowering=False, debug=False)

    actT = nc.dram_tensor("actT", (128, NKT, BC), f8t, kind="ExternalInput").ap()
    wt = nc.dram_tensor("wt", (128, NKT, 512), f8t, kind="ExternalInput").ap()
    povT = nc.dram_tensor("povT", (128, BC), fp32, kind="ExternalInput").ap()
    scales = nc.dram_tensor("scales", (128, 4), fp32, kind="ExternalInput").ap()
    w0t = nc.dram_tensor("w0t", (512, 32), bft, kind="ExternalInput").ap()
    w1t = nc.dram_tensor("w1t", (32, 32), fp32, kind="ExternalInput").ap()
    w2t = nc.dram_tensor("w2t", (32, 1), fp32, kind="ExternalInput").ap()
    b0 = nc.dram_tensor("b0", (32, 1), fp32, kind="ExternalInput").ap()
    b1 = nc.dram_tensor("b1", (32, 1), fp32, kind="ExternalInput").ap()
    b2 = nc.dram_tensor("b2", (1, 1), fp32, kind="ExternalInput").ap()
    out = nc.dram_tensor("out", (1, BC), fp32, kind="ExternalOutput").ap()

    relu = mybir.ActivationFunctionType.Relu
    ident = mybir.ActivationFunctionType.Identity
    copyf = mybir.ActivationFunctionType.Copy
    dr = mybir.MatmulPerfMode.DoubleRow

    with tile.TileContext(nc) as tc:
        with (
            tc.tile_pool(name="consts", bufs=1) as cp,
            tc.tile_pool(name="acts", bufs=8) as ap_,
            tc.tile_pool(name="wts", bufs=8) as wp,
            tc.tile_pool(name="xs", bufs=1) as xp,
            tc.tile_pool(name="tmps", bufs=2) as tp,
            tc.tile_pool(name="psum", bufs=1, space="PSUM") as pp,
        ):
            # constants
            povT_s = cp.tile([128, BC], fp32, tag="povT")
            nc.sync.dma_start(povT_s[:], povT)
            scales_s = cp.tile([128, 4], fp32, tag="scales")
            nc.sync.dma_start(scales_s[:], scales)
            w0t_s = cp.tile([128, 4, 32], bft, tag="w0t")
            nc.sync.dma_start(w0t_s[:], w0t.rearrange("(a p) m -> p a m", p=128))
            w1t_s = cp.tile([32, 32], fp32, tag="w1t")
            nc.sync.dma_start(w1t_s[:], w1t)
            w2t_s = cp.tile([32, 1], fp32, tag="w2t")
            nc.sync.dma_start(w2t_s[:], w2t)
            b0_s = cp.tile([32, 1], fp32, tag="b0")
            nc.sync.dma_start(b0_s[:], b0)
            b1_s = cp.tile([32, 1], fp32, tag="b1")
            nc.sync.dma_start(b1_s[:], b1)
            b2_s = cp.tile([1, 1], fp32, tag="b2")
            nc.sync.dma_start(b2_s[:], b2)

            # PE warm-up during the first stream-DMA window: junk fp32
            # matmuls trip the HAM clock gate to 2.4GHz before the real
            # accumulation starts (~3.4us of sustained work required).
            warm = pp.tile([128, BC], fp32, tag="warm")
            for _ in range(2):
                nc.tensor.matmul(
                    warm[:], povT_s[:, 0:128], povT_s[:], start=True, stop=True
                )

            # psum accumulators: x^T halves [features 128, batch 512]
            # 0: w256[0:128], 1: w256[128:256], 2: b256[0:128], 3: b256[128:256]
            # (biases are folded in via the final all-ones act k-tile)
            acc = [
                pp.tile([128, BC], fp32, tag=f"acc{j}", name=f"acc{j}")
                for j in range(4)
            ]

            # main streaming loop over contraction dim; fp8 DoubleRow
            # consumes k-tile pairs (2 k-planes per cycle).
            kt_done = 0
            g0 = 0
            for gsz in CHUNKS:
                at = ap_.tile([128, G, BC], f8t, tag="at")
                nc.sync.dma_start(at[:, :gsz, :], actT[:, g0 : g0 + gsz, :])
                wtt = wp.tile([128, G, 512], f8t, tag="wtt")
                nc.sync.dma_start(wtt[:, :gsz, :], wt[:, g0 : g0 + gsz, :])
                i = 0
                while i < gsz:
                    first = kt_done == 0
                    if i + 2 <= gsz:
                        last = kt_done + 2 == NKT
                        for j in range(4):
                            nc.tensor.matmul(
                                acc[j][:],
                                wtt[:, i : i + 2, ts(j, 128)],
                                at[:, i : i + 2, :],
                                start=first,
                                stop=last,
                                perf_mode=dr,
                            )
                        kt_done += 2
                        i += 2
                    else:
                        last = kt_done + 1 == NKT
                        for j in range(4):
                            nc.tensor.matmul(
                                acc[j][:],
                                wtt[:, i, ts(j, 128)],
                                at[:, i, :],
                                start=first,
                                stop=last,
                            )
                        kt_done += 1
                        i += 1
                g0 += gsz

            # dequant + pov select + relu, feature-major.
            # x_top = pov ? w256 : b256 ; x_bot = pov ? b256 : w256
            xs = [
                xp.tile([128, BC], bft, tag=f"x{a}", name=f"x{a}")
                for a in range(4)
            ]
            for i in range(2):
                aw = tp.tile([128, BC], fp32, tag="aw")
                nc.scalar.activation(
                    aw[:], acc[i][:], copyf, scale=scales_s[:, i : i + 1]
                )
                ab = tp.tile([128, BC], fp32, tag="ab")
                nc.scalar.activation(
                    ab[:], acc[2 + i][:], copyf, scale=scales_s[:, 2 + i : 3 + i]
                )
                dd = tp.tile([128, BC], fp32, tag="dd")
                nc.vector.tensor_sub(dd[:], aw[:], ab[:])
                pd = tp.tile([128, BC], fp32, tag="pd")
                nc.vector.tensor_mul(pd[:], dd[:], povT_s[:])
                xt = tp.tile([128, BC], fp32, tag="xt")
                nc.vector.tensor_add(xt[:], ab[:], pd[:])
                nc.scalar.activation(xs[i][:], xt[:], relu)
                xb = tp.tile([128, BC], fp32, tag="xb")
                nc.vector.tensor_sub(xb[:], aw[:], pd[:])
                nc.vector.tensor_relu(xs[2 + i][:], xb[:])

            # MLP: 512 -> 32 -> 32 -> 1, fp32
            h0 = pp.tile([32, BC], fp32, tag="h0")
            for a in range(4):
                nc.tensor.matmul(
                    h0[:], w0t_s[:, a, :], xs[a][:], start=(a == 0), stop=(a == 3)
                )
            h0s = tp.tile([32, BC], fp32, tag="h0s")
            nc.scalar.activation(h0s[:], h0[:], relu, bias=b0_s[:])

            h1 = pp.tile([32, BC], fp32, tag="h1")
            nc.tensor.matmul(h1[:], w1t_s[:], h0s[:], start=True, stop=True)
            h1s = tp.tile([32, BC], fp32, tag="h1s")
            nc.scalar.activation(h1s[:], h1[:], relu, bias=b1_s[:])

            y = pp.tile([1, BC], fp32, tag="y")
            nc.tensor.matmul(y[:], w2t_s[:], h1s[:], start=True, stop=True)
            ys = tp.tile([1, BC], fp32, tag="ys")
            nc.scalar.activation(ys[:], y[:], ident, bias=b2_s[:])

            nc.sync.dma_start(out, ys[:])

    _prune_redundant_dma_waits(nc, mybir)
    nc.compile()
    return nc


def _get_compiled():
    global _COMPILED
    if _COMPILED is None:
        _COMPILED = _build()
    return _COMPILED


def kernel(pov, white, black, Ww, bw, Wb, bb, W0, b0, W1, b1, W2, b2):
    global LAST_EXEC_NS, LAST_RESULTS
    from concourse import bass_utils

    pov = np.asarray(pov, np.float32)
    white = np.asarray(white, np.float32)
    black = np.asarray(black, np.float32)
    Ww = np.asarray(Ww, np.float32)
    Wb = np.asarray(Wb, np.float32)

    # Combined feature-transform weights, feature-major [DP, 512].
    # Row D (the all-ones act row) carries the biases.
    Wf = np.zeros((DP, 512), dtype=np.float32)
    Wf[:H, 0:256] = Ww[:, :H].T
    Wf[H:D, 0:256] = Ww[:, H:].T
    Wf[:H, 256:512] = Wb[:, H:].T
    Wf[H:D, 256:512] = Wb[:, :H].T
    Wf[D, 0:256] = np.asarray(bw, np.float32)
    Wf[D, 256:512] = np.asarray(bb, np.float32)

    # fp8 quantization with per-output-column scales
    s = np.abs(Wf).max(axis=0) / F8MAX  # [512]
    s = np.maximum(s, 1e-30)
    Wq = (Wf / s).astype(f8)  # [DP, 512]
    # partition-major layout [128, NKT, 512]
    wt_dev = np.ascontiguousarray(Wq.reshape(NKT, 128, 512).transpose(1, 0, 2))

    scales = np.ascontiguousarray(s.reshape(4, 128).T)  # [128, 4], col j = s[j*128:(j+1)*128]

    whiteb = white.astype(f8)
    blackb = black.astype(f8)

    w0t = np.ascontiguousarray(np.asarray(W0, np.float32).T.astype(bf16))  # [512, 32]
    w1t = np.ascontiguousarray(np.asarray(W1, np.float32).T)  # [32, 32]
    w2t = np.ascontiguousarray(np.asarray(W2, np.float32).T)  # [32, 1]
    b0c = np.asarray(b0, np.float32).reshape(32, 1)
    b1c = np.asarray(b1, np.float32).reshape(32, 1)
    b2c = np.asarray(b2, np.float32).reshape(1, 1)

    in_maps = []
    for c in range(NCORES):
        sl = slice(c * BC, (c + 1) * BC)
        actT = np.zeros((DP, BC), dtype=f8)
        actT[:H] = whiteb[sl].T
        actT[H:D] = blackb[sl].T
        actT[D] = 1.0  # bias row
        act_dev = np.ascontiguousarray(
            actT.reshape(NKT, 128, BC).transpose(1, 0, 2)
        )
        povT = np.ascontiguousarray(
            np.broadcast_to(pov[sl].reshape(1, BC), (128, BC))
        )
        in_maps.append(
            {
                "actT": act_dev,
                "wt": wt_dev,
                "povT": povT,
                "scales": scales,
                "w0t": w0t,
                "w1t": w1t,
                "w2t": w2t,
                "b0": b0c,
                "b1": b1c,
                "b2": b2c,
            }
        )

    nc = _get_compiled()
    res = bass_utils.run_bass_kernel_spmd(
        nc, in_maps, core_ids=list(range(NCORES)), trace=TRACE
    )
    LAST_EXEC_NS = res.exec_time_ns
    LAST_RESULTS = res

    y = np.empty((B, 1), np.float32)
    for c in range(NCORES):
        y[c * BC : (c + 1) * BC, 0] = res.results[c]["out"].reshape(BC)
    return y
